# revision 5
# baseline (speedup 1.0000x reference)
"""Trainium2 Bass kernel for nn_DecoderLayer (self-attn + cross-attn + FFN).

Distribution over 8 NeuronCores: data-parallel over batch (B=2) x
tensor-parallel over heads / rows (4-way). Core c handles batch g=c//4 with
group rank r=c%4 owning heads [4r:4r+4]. One AllReduce after self-attn
out-proj, one ReduceScatter after cross-attn out-proj (each within the
4-core group); the FFN then runs row-parallel on each core's 512-token
slice with full weights, so no further collectives are needed.

Everything on-chip is channel-major ([channel-partition, token] layout);
the host transposes inputs/outputs and pre-transposes/slices the weights.
LayerNorm statistics are computed with ones-vector matmuls on the
TensorEngine (partition-axis reduction); softmax runs on transposed scores
(kv on partitions) so no max-subtraction or probability transposes are
needed; the softmax denominator comes from an extra ones-column appended
to V. All matmuls use the float32r (TF32-like) PE path.
"""
import numpy as np
from contextlib import ExitStack

import concourse.bass as bass
import concourse.tile as tile
from concourse import bacc, mybir
from concourse.bass_utils import run_bass_kernel_spmd

F32 = mybir.dt.float32
F32R = mybir.dt.float32r
AF = mybir.ActivationFunctionType
OP = mybir.AluOpType

P = 128
D = 1024          # model dim
DC = D // P       # 8 channel chunks
LQ = 2048         # query length
LKV = 4096        # kv length
HL = 4            # local heads per core
DH = 64           # head dim
HD = HL * DH      # 256 local projection width
HC = HD // P      # 2 chunks
DF = 4096         # FFN hidden
DFC = DF // P     # 32
NT = 512          # token tile
TQ = LQ // NT     # 4
TKV = LKV // NT   # 8
RT = 512          # per-rank token slice after RS
EPS = 1e-5
RG = [[0, 1, 2, 3], [4, 5, 6, 7]]

_CACHE = {}


def _build():
    nc = bacc.Bacc("TRN2", target_bir_lowering=False, debug=False, num_devices=8)

    def din(name, shape, dt=F32R):
        return nc.dram_tensor(name, shape, dt, kind="ExternalInput").ap()

    qT = din("qT", [D, LQ])
    kT = din("kT", [D, LKV])
    vT = din("vT", [D, LKV])
    wq_sa = din("wq_sa", [D, HD])
    wk_sa = din("wk_sa", [D, HD])
    wv_sa = din("wv_sa", [D, HD])
    wo_sa = din("wo_sa", [HD, D])
    wq_ca = din("wq_ca", [D, HD])
    wk_ca = din("wk_ca", [D, HD])
    wv_ca = din("wv_ca", [D, HD])
    wo_ca = din("wo_ca", [HD, D])
    w1 = din("w1", [D, DF])
    w2 = din("w2", [DF, D])
    b1f = din("b1f", [P, DFC], F32)
    b2f = din("b2f", [P, DC], F32)
    resg_sa = din("resg_sa", [P, DC], F32)
    resg_ca = din("resg_ca", [P, DC], F32)
    wfs_k = din("wfs_k", [P, HC], F32)
    wfs_v = din("wfs_v", [P, HC], F32)
    ones4 = din("ones4", [P, HL])
    onesr = din("onesr", [1, P])
    ident = din("ident", [P, P])

    outT = nc.dram_tensor("outT", [D, RT], F32, kind="ExternalOutput").ap()

    # internal DRAM
    ar1_in = nc.dram_tensor("ar1_in", [D, LQ], F32R).ap()
    ar1_out = nc.dram_tensor("ar1_out", [D, LQ], F32R).ap()
    xh2_d = nc.dram_tensor("xh2_d", [D, LQ], F32R).ap()
    rs2_in = nc.dram_tensor("rs2_in", [TQ, D, NT], F32R).ap()
    rs2_out = nc.dram_tensor("rs2_out", [D, RT], F32R).ap()

    def cm(ap):  # [C*P, L] -> [P, C, L]
        return ap.rearrange("(c p) l -> p c l", p=P)

    qT_v, kT_v, vT_v = cm(qT), cm(kT), cm(vT)
    ar1_in_v, ar1_out_v, xh2_v = cm(ar1_in), cm(ar1_out), cm(xh2_d)
    rs2_in_v = rs2_in.rearrange("t (c p) l -> p t c l", p=P)
    rs2_out_v, outT_v = cm(rs2_out), cm(outT)
    w1_v = cm(w1)   # [P, 8, 4096]
    w2_v = cm(w2)   # [P, 32, 1024]

    ts = bass.ts

    with tile.TileContext(nc) as tc:
        ctx = ExitStack()
        with ctx:
            constp = ctx.enter_context(tc.tile_pool(name="const", bufs=1))
            statp1 = ctx.enter_context(tc.tile_pool(name="stats1", bufs=1))
            statp = ctx.enter_context(tc.tile_pool(name="stats2", bufs=2))
            work = ctx.enter_context(tc.tile_pool(name="work", bufs=2))

            ones4_t = constp.tile([P, HL], F32R)
            nc.sync.dma_start(ones4_t, ones4)
            onesr_t = constp.tile([1, P], F32R)
            nc.sync.dma_start(onesr_t, onesr)
            ident_t = constp.tile([P, P], F32R)
            nc.sync.dma_start(ident_t, ident)
            eps_t = constp.tile([1, 1], F32)
            nc.vector.memset(eps_t, EPS)
            b1f_t = constp.tile([P, DFC], F32)
            nc.sync.dma_start(b1f_t, b1f)
            b2f_t = constp.tile([P, DC], F32)
            nc.sync.dma_start(b2f_t, b2f)
            resg_sa_t = constp.tile([P, DC], F32)
            nc.sync.dma_start(resg_sa_t, resg_sa)
            resg_ca_t = constp.tile([P, DC], F32)
            nc.sync.dma_start(resg_ca_t, resg_ca)
            wfs_k_t = constp.tile([P, HC], F32)
            nc.sync.dma_start(wfs_k_t, wfs_k)
            wfs_v_t = constp.tile([P, HC], F32)
            nc.sync.dma_start(wfs_v_t, wfs_v)

            def ln_stats(blk, C, pst):
                """blk [P, C, 512] F32R -> (mu [1,512] F32R, rstd [1,512] F32R)."""
                ps_sum = pst.tile([1, NT], F32, tag="ps_sum")
                ps_sq = pst.tile([1, NT], F32, tag="ps_sq")
                for c in range(C):
                    xsq = work.tile([P, NT], F32R, tag="xsq")
                    nc.vector.tensor_mul(xsq, blk[:, c, :], blk[:, c, :])
                    nc.tensor.matmul(ps_sum, ones4_t[:, 0:1], blk[:, c, :],
                                     start=(c == 0), stop=(c == C - 1))
                    nc.tensor.matmul(ps_sq, ones4_t[:, 0:1], xsq,
                                     start=(c == 0), stop=(c == C - 1))
                mu = statp.tile([1, NT], F32R, tag="mu")
                nc.vector.tensor_scalar_mul(mu, ps_sum, 1.0 / D)
                ex2 = statp1.tile([1, NT], F32, tag="ex2")
                nc.vector.tensor_scalar_mul(ex2, ps_sq, 1.0 / D)
                musq = statp1.tile([1, NT], F32, tag="musq")
                nc.vector.tensor_mul(musq, mu, mu)
                var = statp1.tile([1, NT], F32, tag="var")
                nc.vector.tensor_tensor(var, ex2, musq, OP.subtract)
                std = statp1.tile([1, NT], F32, tag="std")
                nc.scalar.activation(std, var, AF.Sqrt, bias=eps_t)
                rstd = statp.tile([1, NT], F32R, tag="rstd")
                with nc.allow_low_precision(reason="rstd in f32r for PE bcast"):
                    nc.vector.reciprocal(rstd, std)
                return mu, rstd

            def bcast(row, pool, tag):
                """[1,512] F32R -> psum [P,512] F32 replicated across partitions."""
                ps = pool.tile([P, NT], F32, tag=tag)
                nc.tensor.matmul(ps, onesr_t, row, start=True, stop=True)
                return ps

            def ln_materialize(blk, dst, tslice, C, pst, pbc):
                """dst[:, c, tslice] = (blk - mu) * rstd (channel-major LN)."""
                mu, rstd = ln_stats(blk, C, pst)
                mu_b = bcast(mu, pbc, "bc0")
                rstd_b = bcast(rstd, pbc, "bc1")
                for c in range(C):
                    tmp = work.tile([P, NT], F32, tag="lnt")
                    nc.vector.tensor_tensor(tmp, blk[:, c, :], mu_b, OP.subtract)
                    nc.vector.tensor_tensor(dst[:, c, tslice], tmp, rstd_b, OP.mult)

            # ---------------- Phase A: LN(q) -> xh1 (resident A..B3) -------------
            selfblk = ExitStack()
            ctx.enter_context(selfblk)
            bigp = selfblk.enter_context(tc.tile_pool(name="big", bufs=1))
            xh1 = bigp.tile([P, DC, LQ], F32R, tag="xh1")
            with tc.tile_pool(name="qraw", bufs=2) as qrawp, \
                 tc.tile_pool(name="psA", bufs=1, space="PSUM") as psA, \
                 tc.tile_pool(name="pbA", bufs=1, space="PSUM") as pbA:
                for t in range(TQ):
                    blk = qrawp.tile([P, DC, NT], F32R, tag="qblk")
                    nc.sync.dma_start(blk, qT_v[:, :, ts(t, NT)])
                    ln_materialize(blk, xh1, ts(t, NT), DC, psA, pbA)

            def project(src_full, w_t, m_chunks, L, pmm, evict):
                """evict(psum, m, t) over sum_c w_t[:,c,m-tile].T @ src[:,c,t-tile]."""
                for t in range(L // NT):
                    for m in range(m_chunks):
                        ps = pmm.tile([P, NT], F32, tag="proj")
                        for c in range(DC):
                            nc.tensor.matmul(ps, w_t[:, c, ts(m, P)],
                                             src_full[:, c, ts(t, NT)],
                                             start=(c == 0), stop=(c == DC - 1))
                        evict(ps, m, t)

            def transpose_split(vtile, dst_aug, t, m, ptp):  # vtile [P,512]
                """[P,512] channel-major V tile -> token-major head slabs."""
                for j in range(NT // P):
                    kt = t * (NT // P) + j
                    tp = ptp.tile([P, P], F32R, tag="vtp")
                    nc.tensor.transpose(tp, vtile[:, ts(j, P)], ident_t)
                    nc.vector.tensor_copy(dst_aug[:, kt, 2 * m, 0:DH], tp[:, 0:DH])
                    nc.vector.tensor_copy(dst_aug[:, kt, 2 * m + 1, 0:DH],
                                          tp[:, DH:P])

            def flash(qT_x, khT_x, vh_aug, oT_x, Lkv, psf, pso, fwork):
                KT = Lkv // P
                for h in range(HL):
                    hp, hc = DH * (h % 2), h // 2
                    for s in range(TQ):
                        o_ps = pso.tile([DH + 1, NT], F32, tag="o_ps")
                        for kt in range(KT):
                            s_ps = psf.tile([P, NT], F32, tag="s_ps")
                            nc.tensor.matmul(
                                s_ps,
                                khT_x[hp:hp + DH, hc, ts(kt, P)],
                                qT_x[hp:hp + DH, hc, ts(s, NT)],
                                start=True, stop=True)
                            probs = fwork.tile([P, NT], F32R, tag="probs")
                            nc.scalar.activation(probs, s_ps, AF.Exp)
                            nc.tensor.matmul(o_ps, vh_aug[:, kt, h, :], probs,
                                             start=(kt == 0), stop=(kt == KT - 1))
                        recip = statp.tile([1, NT], F32R, tag="recip")
                        with nc.allow_low_precision(
                                reason="softmax denom recip f32r for PE bcast"):
                            nc.vector.reciprocal(recip, o_ps[DH:DH + 1, :])
                        r_b = bcast(recip, pso, "r_b")
                        o_sb = fwork.tile([DH, NT], F32, tag="o_sb")
                        nc.vector.tensor_copy(o_sb, o_ps[0:DH, :])
                        nc.vector.tensor_tensor(oT_x[hp:hp + DH, hc, ts(s, NT)],
                                                o_sb, r_b[0:DH, :], OP.mult)

            def out_proj(oT_x, wo_t, pmm, dst_fn):
                for m in range(DC):
                    for t in range(TQ):
                        ps = pmm.tile([P, NT], F32, tag="proj")
                        for hc in range(HC):
                            nc.tensor.matmul(ps, wo_t[:, hc, ts(m, P)],
                                             oT_x[:, hc, ts(t, NT)],
                                             start=(hc == 0), stop=(hc == HC - 1))
                        dst_fn(ps, m, t)

            # ---------------- Phase B: self-attention ----------------
            sattn = ExitStack()
            with sattn:
                sawp = sattn.enter_context(tc.tile_pool(name="saw", bufs=1))
                sap = sattn.enter_context(tc.tile_pool(name="sattn", bufs=1))

                wq_sa_t = sawp.tile([P, DC, HD], F32R, tag="wq")
                nc.sync.dma_start(wq_sa_t, cm(wq_sa))
                wk_sa_t = sawp.tile([P, DC, HD], F32R, tag="wk")
                nc.sync.dma_start(wk_sa_t, cm(wk_sa))
                wv_sa_t = sawp.tile([P, DC, HD], F32R, tag="wv")
                nc.sync.dma_start(wv_sa_t, cm(wv_sa))

                qTs = sap.tile([P, HC, LQ], F32R, tag="qTs")
                khTs = sap.tile([P, HC, LQ], F32R, tag="khTs")
                vh_sa = sap.tile([P, LQ // P, HL, DH + 1], F32R, tag="vh_sa")

                with tc.tile_pool(name="psB1", bufs=2, space="PSUM") as psB1, \
                     tc.tile_pool(name="ptB1", bufs=2, space="PSUM") as ptB1, \
                     tc.tile_pool(name="wkB1", bufs=2) as wkB1:
                    project(xh1, wq_sa_t, HC, LQ, psB1,
                            lambda ps, m, t: nc.vector.tensor_copy(
                                qTs[:, m, ts(t, NT)], ps))
                    project(xh1, wk_sa_t, HC, LQ, psB1,
                            lambda ps, m, t: nc.vector.tensor_copy(
                                khTs[:, m, ts(t, NT)], ps))

                    def v_sa_evict(ps, m, t):
                        vtile = wkB1.tile([P, NT], F32R, tag="vtile")
                        nc.vector.tensor_copy(vtile, ps)
                        transpose_split(vtile, vh_sa, t, m, ptB1)

                    project(xh1, wv_sa_t, HC, LQ, psB1, v_sa_evict)
                    for kt in range(LQ // P):
                        nc.sync.dma_start(vh_sa[:, kt, :, DH], ones4)

                oTs = sap.tile([P, HC, LQ], F32R, tag="oTs")
                with tc.tile_pool(name="psf", bufs=3, space="PSUM") as psf, \
                     tc.tile_pool(name="pso", bufs=2, space="PSUM") as pso, \
                     tc.tile_pool(name="fwk", bufs=3) as fwk:
                    flash(qTs, khTs, vh_sa, oTs, LQ, psf, pso, fwk)

                wo_sa_t = sawp.tile([P, HC, D], F32R, tag="wq")
                nc.sync.dma_start(wo_sa_t, cm(wo_sa))

                def ar1_dst(ps, m, t):
                    stage = wkB3.tile([P, NT], F32R, tag="ar1st")
                    nc.vector.scalar_tensor_tensor(
                        stage, xh1[:, m, ts(t, NT)], resg_sa_t[:, m:m + 1], ps,
                        OP.mult, OP.add)
                    nc.sync.dma_start(ar1_in_v[:, m, ts(t, NT)], stage)

                with tc.tile_pool(name="psB3", bufs=2, space="PSUM") as psB3, \
                     tc.tile_pool(name="wkB3", bufs=2) as wkB3:
                    out_proj(oTs, wo_sa_t, psB3, ar1_dst)

            selfblk.close()   # frees xh1
            nc.gpsimd.collective_compute(
                "AllReduce", OP.add, replica_groups=RG,
                ins=[ar1_in], outs=[ar1_out])

            # ------------- Phase B': cross K/V (overlaps AR1) -------------
            cattn = ExitStack()
            with cattn:
                cawp = cattn.enter_context(tc.tile_pool(name="caw", bufs=1))
                cap = cattn.enter_context(tc.tile_pool(name="cattn", bufs=1))

                wk_ca_t = cawp.tile([P, DC, HD], F32R, tag="wkc")
                nc.sync.dma_start(wk_ca_t, cm(wk_ca))
                wv_ca_t = cawp.tile([P, DC, HD], F32R, tag="wvc")
                nc.sync.dma_start(wv_ca_t, cm(wv_ca))

                khTc = cap.tile([P, HC, LKV], F32R, tag="khTc")
                vh_ca = cap.tile([P, LKV // P, HL, DH + 1], F32R, tag="vh_ca")

                with tc.tile_pool(name="kvblk", bufs=2) as kvp, \
                     tc.tile_pool(name="psBp", bufs=1, space="PSUM") as psBp, \
                     tc.tile_pool(name="pbBp", bufs=1, space="PSUM") as pbBp, \
                     tc.tile_pool(name="pjBp", bufs=2, space="PSUM") as pjBp, \
                     tc.tile_pool(name="ptBp", bufs=1, space="PSUM") as ptBp, \
                     tc.tile_pool(name="wkBp", bufs=2) as wkBp:

                    def fused_ln_project(src_v, w_t, wfs_t, t, dst_evict):
                        blk = kvp.tile([P, DC, NT], F32R, tag="kvb")
                        nc.sync.dma_start(blk, src_v[:, :, ts(t, NT)])
                        mu, rstd = ln_stats(blk, DC, psBp)
                        negmr = statp.tile([1, NT], F32R, tag="negmr")
                        nc.vector.tensor_mul(negmr, mu, rstd)
                        nc.vector.tensor_scalar_mul(negmr, negmr, -1.0)
                        a_ps = bcast(rstd, pbBp, "bc0")
                        c_ps = bcast(negmr, pbBp, "bc1")
                        a_sb = wkBp.tile([P, NT], F32, tag="a_sb")
                        nc.vector.tensor_copy(a_sb, a_ps)
                        c_sb = wkBp.tile([P, NT], F32, tag="c_sb")
                        nc.vector.tensor_copy(c_sb, c_ps)
                        for m in range(HC):
                            ps = pjBp.tile([P, NT], F32, tag="proj")
                            for c in range(DC):
                                nc.tensor.matmul(ps, w_t[:, c, ts(m, P)],
                                                 blk[:, c, :],
                                                 start=(c == 0), stop=(c == DC - 1))
                            t1 = wkBp.tile([P, NT], F32, tag="t1")
                            nc.vector.tensor_mul(t1, ps, a_sb)
                            dst_evict(t1, c_sb, wfs_t, m, t)

                    def kh_evict(t1, c_sb, wfs_t, m, t):
                        nc.vector.scalar_tensor_tensor(
                            khTc[:, m, ts(t, NT)], c_sb, wfs_t[:, m:m + 1], t1,
                            OP.mult, OP.add)

                    def vh_evict(t1, c_sb, wfs_t, m, t):
                        vtile = wkBp.tile([P, NT], F32R, tag="vtile")
                        nc.vector.scalar_tensor_tensor(
                            vtile, c_sb, wfs_t[:, m:m + 1], t1, OP.mult, OP.add)
                        transpose_split(vtile, vh_ca, t, m, ptBp)

                    for t in range(TKV):
                        fused_ln_project(kT_v, wk_ca_t, wfs_k_t, t, kh_evict)
                        fused_ln_project(vT_v, wv_ca_t, wfs_v_t, t, vh_evict)
                    for kt in range(LKV // P):
                        nc.sync.dma_start(vh_ca[:, kt, :, DH], ones4)

                # ------------- Phase C: q2 LN + cross Q proj (spill xh2) ---------
                wq_ca_t = cawp.tile([P, DC, HD], F32R, tag="wqc")
                nc.sync.dma_start(wq_ca_t, cm(wq_ca))
                qTc = cap.tile([P, HC, LQ], F32R, tag="qTc")
                with tc.tile_pool(name="q2raw", bufs=1) as q2p, \
                     tc.tile_pool(name="psC", bufs=1, space="PSUM") as psC, \
                     tc.tile_pool(name="pbC", bufs=1, space="PSUM") as pbC, \
                     tc.tile_pool(name="pjC", bufs=2, space="PSUM") as pjC:
                    for t in range(TQ):
                        blk = q2p.tile([P, DC, NT], F32R, tag="q2blk")
                        nc.sync.dma_start(blk, ar1_out_v[:, :, ts(t, NT)])
                        xh2b = q2p.tile([P, DC, NT], F32R, tag="xh2b")
                        ln_materialize(blk, xh2b, slice(0, NT), DC, psC, pbC)
                        nc.sync.dma_start(xh2_v[:, :, ts(t, NT)], xh2b)
                        for m in range(HC):
                            ps = pjC.tile([P, NT], F32, tag="proj")
                            for c in range(DC):
                                nc.tensor.matmul(ps, wq_ca_t[:, c, ts(m, P)],
                                                 xh2b[:, c, :],
                                                 start=(c == 0), stop=(c == DC - 1))
                            nc.vector.tensor_copy(qTc[:, m, ts(t, NT)], ps)

                # ------------- Phase D: cross flash + out-proj -> RS2 -------------
                oTc = cap.tile([P, HC, LQ], F32R, tag="oTc")
                with tc.tile_pool(name="psfD", bufs=3, space="PSUM") as psfD, \
                     tc.tile_pool(name="psoD", bufs=2, space="PSUM") as psoD, \
                     tc.tile_pool(name="fwkD", bufs=3) as fwkD:
                    flash(qTc, khTc, vh_ca, oTc, LKV, psfD, psoD, fwkD)

                wo_ca_t = cawp.tile([P, HC, D], F32R, tag="wkc")
                nc.sync.dma_start(wo_ca_t, cm(wo_ca))

                with tc.tile_pool(name="psD2", bufs=2, space="PSUM") as psD2, \
                     tc.tile_pool(name="wkD2", bufs=2) as wkD2:
                    def rs2_dst(ps, m, t):
                        xh2_t = wkD2.tile([P, NT], F32R, tag="xh2r")
                        nc.sync.dma_start(xh2_t, xh2_v[:, m, ts(t, NT)])
                        stage = wkD2.tile([P, NT], F32R, tag="rs2st")
                        nc.vector.scalar_tensor_tensor(
                            stage, xh2_t, resg_ca_t[:, m:m + 1], ps, OP.mult, OP.add)
                        nc.sync.dma_start(rs2_in_v[:, t, m, :], stage)

                    out_proj(oTc, wo_ca_t, psD2, rs2_dst)

            nc.gpsimd.collective_compute(
                "ReduceScatter", OP.add, replica_groups=RG,
                ins=[rs2_in], outs=[rs2_out])

            # ---------------- Phase E: FFN on the 512-token slice ----------------
            ffn = ExitStack()
            with ffn:
                fp = ffn.enter_context(tc.tile_pool(name="ffn", bufs=1))
                w1p = ffn.enter_context(tc.tile_pool(name="w1p", bufs=4))
                w2p = ffn.enter_context(tc.tile_pool(name="w2p", bufs=2))
                psE = ffn.enter_context(tc.tile_pool(name="psE", bufs=1, space="PSUM"))
                pbE = ffn.enter_context(tc.tile_pool(name="pbE", bufs=1, space="PSUM"))
                pjE = ffn.enter_context(tc.tile_pool(name="pjE", bufs=3, space="PSUM"))
                ework = ffn.enter_context(tc.tile_pool(name="ework", bufs=2))

                x3 = fp.tile([P, DC, RT], F32R, tag="x3")
                nc.sync.dma_start(x3, rs2_out_v)
                xh3 = fp.tile([P, DC, RT], F32R, tag="xh3")
                ln_materialize(x3, xh3, slice(0, RT), DC, psE, pbE)

                hT = fp.tile([P, DFC, RT], F32R, tag="hT")
                for m in range(DFC):
                    w1t = w1p.tile([P, DC, P], F32R, tag="w1t")
                    nc.sync.dma_start(w1t, w1_v[:, :, ts(m, P)])
                    ps = pjE.tile([P, RT], F32, tag="proj")
                    for c in range(DC):
                        nc.tensor.matmul(ps, w1t[:, c, :], xh3[:, c, :],
                                         start=(c == 0), stop=(c == DC - 1))
                    nc.scalar.activation(hT[:, m, :], ps, AF.Relu,
                                         bias=b1f_t[:, m:m + 1])

                for m2 in range(DC):
                    w2t = w2p.tile([P, DFC, P], F32R, tag="w2t")
                    nc.sync.dma_start(w2t, w2_v[:, :, ts(m2, P)])
                    ps = pjE.tile([P, RT], F32, tag="proj")
                    for c in range(DFC):
                        nc.tensor.matmul(ps, w2t[:, c, :], hT[:, c, :],
                                         start=(c == 0), stop=(c == DFC - 1))
                    stage = ework.tile([P, RT], F32, tag="outst")
                    nc.vector.scalar_tensor_tensor(
                        stage, ps, b2f_t[:, m2:m2 + 1], x3[:, m2, :],
                        OP.add, OP.add)
                    nc.sync.dma_start(outT_v[:, m2, :], stage)

    nc.compile()
    return nc


def _host_inputs(inputs):
    """Build the 8 per-core input maps from the full problem inputs."""
    f = np.float32
    q = np.asarray(inputs["q"], f)
    k = np.asarray(inputs["k"], f)
    v = np.asarray(inputs["v"], f)
    g_sa = np.asarray(inputs["g_sa"], f)
    b_sa = np.asarray(inputs["b_sa"], f)
    g_qca = np.asarray(inputs["g_qca"], f)
    b_qca = np.asarray(inputs["b_qca"], f)
    g_kca = np.asarray(inputs["g_kca"], f)
    b_kca = np.asarray(inputs["b_kca"], f)
    g_vca = np.asarray(inputs["g_vca"], f)
    b_vca = np.asarray(inputs["b_vca"], f)
    g_out = np.asarray(inputs["g_out"], f)
    b_out = np.asarray(inputs["b_out"], f)
    for name, b in [("b_sa", b_sa), ("b_qca", b_qca), ("b_kca", b_kca),
                    ("b_vca", b_vca)]:
        assert np.all(b == 0), f"{name} nonzero not supported"

    sa_wq = np.asarray(inputs["sa_wq"], f) * g_sa[None, :] * 0.125
    sa_wk = np.asarray(inputs["sa_wk"], f) * g_sa[None, :]
    sa_wv = np.asarray(inputs["sa_wv"], f) * g_sa[None, :]
    sa_wo = np.asarray(inputs["sa_wo"], f)
    ca_wq = np.asarray(inputs["ca_wq"], f) * g_qca[None, :] * 0.125
    ca_wk = np.asarray(inputs["ca_wk"], f) * g_kca[None, :]
    ca_wv = np.asarray(inputs["ca_wv"], f) * g_vca[None, :]
    ca_wo = np.asarray(inputs["ca_wo"], f)
    w1f = (np.asarray(inputs["ffn_w1"], f) * g_out[None, :]).T.copy()
    w2f = np.asarray(inputs["ffn_w2"], f).T.copy()
    b1f_vec = np.asarray(inputs["ffn_w1"], f) @ b_out + np.asarray(inputs["ffn_b1"], f)
    b2_vec = np.asarray(inputs["ffn_b2"], f)

    def chunked(vec):  # [C*P] -> [P, C]
        return np.ascontiguousarray(vec.reshape(-1, P).T)

    in_maps = []
    for c in range(8):
        g, r = c // 4, c % 4
        hs = slice(r * HD, (r + 1) * HD)
        flag = 1.0 if r == 0 else 0.0
        wk_eff = ca_wk[hs]
        wv_eff = ca_wv[hs]
        in_maps.append({
            "qT": np.ascontiguousarray(q[g].T),
            "kT": np.ascontiguousarray(k[g].T),
            "vT": np.ascontiguousarray(v[g].T),
            "wq_sa": np.ascontiguousarray(sa_wq[hs].T),
            "wk_sa": np.ascontiguousarray(sa_wk[hs].T),
            "wv_sa": np.ascontiguousarray(sa_wv[hs].T),
            "wo_sa": np.ascontiguousarray(sa_wo[:, hs].T),
            "wq_ca": np.ascontiguousarray(ca_wq[hs].T),
            "wk_ca": np.ascontiguousarray(wk_eff.T),
            "wv_ca": np.ascontiguousarray(wv_eff.T),
            "wo_ca": np.ascontiguousarray(ca_wo[:, hs].T),
            "w1": w1f,
            "w2": w2f,
            "b1f": chunked(b1f_vec),
            "b2f": chunked(b2_vec),
            "resg_sa": chunked(flag * g_sa),
            "resg_ca": chunked(flag * g_qca),
            "wfs_k": chunked(wk_eff.sum(axis=1)),
            "wfs_v": chunked(wv_eff.sum(axis=1)),
            "ones4": np.ones((P, HL), f),
            "onesr": np.ones((1, P), f),
            "ident": np.eye(P, dtype=f),
        })
    return in_maps


def _run(inputs, trace=False, trace_cores=None):
    if "nc" not in _CACHE:
        _CACHE["nc"] = _build()
    nc = _CACHE["nc"]
    in_maps = _host_inputs(inputs)
    res = run_bass_kernel_spmd(nc, in_maps, core_ids=list(range(8)),
                               trace=trace, trace_cores=trace_cores)
    out = np.empty((2, LQ, D), np.float32)
    for c in range(8):
        g, r = c // 4, c % 4
        out[g, r * RT:(r + 1) * RT, :] = res.results[c]["outT"].T
    return out, res


def kernel(**inputs):
    out, _ = _run(inputs)
    return out


# revision 17
# speedup vs baseline: 1.1391x; 1.1391x over previous
"""Trainium2 Bass kernel for nn_DecoderLayer (self-attn + cross-attn + FFN).

Distribution over 8 NeuronCores: data-parallel over batch (B=2) x
tensor-parallel over heads / rows (4-way). Core c handles batch g=c//4 with
group rank r=c%4 owning heads [4r:4r+4]. One AllReduce after self-attn
out-proj, one ReduceScatter after cross-attn out-proj (each within the
4-core group); the FFN then runs row-parallel on each core's 512-token
slice with full weights, so no further collectives are needed.

Everything on-chip is channel-major ([channel-partition, token] layout);
the host transposes inputs/outputs and pre-transposes/slices the weights.
LayerNorm statistics are computed with ones-vector matmuls on the
TensorEngine (partition-axis reduction); softmax runs on transposed scores
(kv on partitions) so no max-subtraction or probability transposes are
needed; the softmax denominator comes from an extra ones-column appended
to V. All matmuls use the float32r (TF32-like) PE path.
"""
import numpy as np
from contextlib import ExitStack

import concourse.bass as bass
import concourse.tile as tile
from concourse import bacc, mybir
from concourse.bass_utils import run_bass_kernel_spmd

F32 = mybir.dt.float32
F32R = mybir.dt.float32r
BF16 = mybir.dt.bfloat16
AF = mybir.ActivationFunctionType
OP = mybir.AluOpType

P = 128
D = 1024          # model dim
DC = D // P       # 8 channel chunks
LQ = 2048         # query length
LKV = 4096        # kv length
HL = 4            # local heads per core
DH = 64           # head dim
HD = HL * DH      # 256 local projection width
HC = HD // P      # 2 chunks
DF = 4096         # FFN hidden
DFC = DF // P     # 32
NT = 512          # token tile
TQ = LQ // NT     # 4
TKV = LKV // NT   # 8
RT = 512          # per-rank token slice after RS
EPS = 1e-5
RG = [[0, 1, 2, 3], [4, 5, 6, 7]]

_CACHE = {}


def _build():
    nc = bacc.Bacc("TRN2", target_bir_lowering=False, debug=False, num_devices=8)

    def din(name, shape, dt=F32R):
        return nc.dram_tensor(name, shape, dt, kind="ExternalInput").ap()

    qT = din("qT", [D, LQ])
    kT = din("kT", [D, LKV])
    vT = din("vT", [D, LKV])
    wq_sa = din("wq_sa", [D, HD])
    wk_sa = din("wk_sa", [D, HD])
    wv_sa = din("wv_sa", [D, HD])
    wo_sa = din("wo_sa", [HD, D])
    wq_ca = din("wq_ca", [D, HD])
    wk_ca = din("wk_ca", [D, HD])
    wv_ca = din("wv_ca", [D, HD])
    wo_ca = din("wo_ca", [HD, D])
    w1 = din("w1", [D, DF], BF16)
    w2 = din("w2", [DF, D], BF16)
    b1f = din("b1f", [P, DFC], F32)
    b2f = din("b2f", [P, DC], F32)
    resg_sa = din("resg_sa", [P, DC], F32)
    resg_ca = din("resg_ca", [P, DC], F32)
    wfs_k = din("wfs_k", [P, HC], F32)
    wfs_v = din("wfs_v", [P, HC], F32)
    ones4 = din("ones4", [P, HL])
    onesr = din("onesr", [1, P])
    onesq = din("onesq", [65, P])
    ident = din("ident", [P, P])

    outT = nc.dram_tensor("outT", [D, RT], F32, kind="ExternalOutput").ap()

    # internal DRAM
    ar1_in = nc.dram_tensor("ar1_in", [D, LQ], F32R).ap()
    ar1_out = nc.dram_tensor("ar1_out", [D, LQ], F32R).ap()
    xh2_d = nc.dram_tensor("xh2_d", [D, LQ], F32R).ap()
    rs2_in = nc.dram_tensor("rs2_in", [TQ, D, NT], F32R).ap()
    rs2_out = nc.dram_tensor("rs2_out", [D, RT], F32R).ap()

    def cm(ap):  # [C*P, L] -> [P, C, L]
        return ap.rearrange("(c p) l -> p c l", p=P)

    qT_v, kT_v, vT_v = cm(qT), cm(kT), cm(vT)
    ar1_in_v, ar1_out_v, xh2_v = cm(ar1_in), cm(ar1_out), cm(xh2_d)
    rs2_in_v = rs2_in.rearrange("t (c p) l -> p t c l", p=P)
    rs2_out_v, outT_v = cm(rs2_out), cm(outT)
    w1_v = cm(w1)   # [P, 8, 4096]
    w2_v = cm(w2)   # [P, 32, 1024]

    ts = bass.ts

    with tile.TileContext(nc) as tc:
        ctx = ExitStack()
        with ctx:
            constp = ctx.enter_context(tc.tile_pool(name="const", bufs=1))
            statp1 = ctx.enter_context(tc.tile_pool(name="stats1", bufs=1))
            statp = ctx.enter_context(tc.tile_pool(name="stats2", bufs=2))
            work = ctx.enter_context(tc.tile_pool(name="work", bufs=2))

            ones4_t = constp.tile([P, HL], F32R)
            nc.sync.dma_start(ones4_t, ones4)
            onesr_t = constp.tile([1, P], F32R)
            nc.sync.dma_start(onesr_t, onesr)
            onesq_t = constp.tile([65, P], F32R)
            nc.sync.dma_start(onesq_t, onesq)
            ident_t = constp.tile([P, P], F32R)
            nc.sync.dma_start(ident_t, ident)
            eps_t = constp.tile([1, 1], F32)
            nc.vector.memset(eps_t, EPS)
            b1f_t = constp.tile([P, DFC], F32)
            nc.sync.dma_start(b1f_t, b1f)
            b2f_t = constp.tile([P, DC], F32)
            nc.sync.dma_start(b2f_t, b2f)
            resg_sa_t = constp.tile([P, DC], F32)
            nc.sync.dma_start(resg_sa_t, resg_sa)
            resg_ca_t = constp.tile([P, DC], F32)
            nc.sync.dma_start(resg_ca_t, resg_ca)
            wfs_k_t = constp.tile([P, HC], F32)
            nc.sync.dma_start(wfs_k_t, wfs_k)
            wfs_v_t = constp.tile([P, HC], F32)
            nc.sync.dma_start(wfs_v_t, wfs_v)

            def ln_stats(blk, C, pst):
                """blk [P, C, 512] F32R -> (mu [1,512] F32R, rstd [1,512] F32R)."""
                ps_sum = pst.tile([1, NT], F32, tag="ps_sum")
                ps_sq = pst.tile([1, NT], F32, tag="ps_sq")
                for c in range(C):
                    xsq = work.tile([P, NT], F32R, tag="xsq")
                    nc.vector.tensor_mul(xsq, blk[:, c, :], blk[:, c, :])
                    nc.tensor.matmul(ps_sum, ones4_t[:, 0:1], blk[:, c, :],
                                     start=(c == 0), stop=(c == C - 1))
                    nc.tensor.matmul(ps_sq, ones4_t[:, 0:1], xsq,
                                     start=(c == 0), stop=(c == C - 1))
                mu = statp.tile([1, NT], F32R, tag="mu")
                with nc.allow_low_precision(reason="ln moments f32r"):
                    nc.vector.tensor_scalar_mul(mu, ps_sum, 1.0 / D)
                ex2 = statp1.tile([1, NT], F32, tag="ex2")
                nc.vector.tensor_scalar_mul(ex2, ps_sq, 1.0 / D)
                musq = statp1.tile([1, NT], F32, tag="musq")
                nc.vector.tensor_mul(musq, mu, mu)
                var = statp1.tile([1, NT], F32, tag="var")
                nc.vector.tensor_tensor(var, ex2, musq, OP.subtract)
                std = statp1.tile([1, NT], F32, tag="std")
                nc.scalar.activation(std, var, AF.Sqrt, bias=eps_t)
                rstd = statp.tile([1, NT], F32R, tag="rstd")
                with nc.allow_low_precision(reason="rstd in f32r for PE bcast"):
                    nc.vector.reciprocal(rstd, std)
                return mu, rstd

            def bcast(row, pool, tag):
                """[1,512] F32R -> psum [P,512] F32 replicated across partitions."""
                ps = pool.tile([P, NT], F32, tag=tag)
                nc.tensor.matmul(ps, onesr_t, row, start=True, stop=True)
                return ps

            def ln_materialize(blk, dst, tslice, C, pst, pbc):
                """dst[:, c, tslice] = (blk - mu) * rstd (channel-major LN)."""
                mu, rstd = ln_stats(blk, C, pst)
                mu_b = bcast(mu, pbc, "bc0")
                rstd_b = bcast(rstd, pbc, "bc1")
                for c in range(C):
                    tmp = work.tile([P, NT], F32, tag="lnt")
                    nc.vector.tensor_tensor(tmp, blk[:, c, :], mu_b, OP.subtract)
                    nc.vector.tensor_tensor(dst[:, c, tslice], tmp, rstd_b, OP.mult)

            # ---------------- Phase A: LN(q) -> xh1 (resident A..B3) -------------
            selfblk = ExitStack()
            ctx.enter_context(selfblk)
            bigp = selfblk.enter_context(tc.tile_pool(name="big", bufs=1))
            xh1 = bigp.tile([P, DC, LQ], F32R, tag="xh1")
            with tc.tile_pool(name="qraw", bufs=2) as qrawp, \
                 tc.tile_pool(name="psA", bufs=1, space="PSUM") as psA, \
                 tc.tile_pool(name="pbA", bufs=1, space="PSUM") as pbA:
                for t in range(TQ):
                    blk = qrawp.tile([P, DC, NT], F32R, tag="qblk")
                    nc.sync.dma_start(blk, qT_v[:, :, ts(t, NT)])
                    ln_materialize(blk, xh1, ts(t, NT), DC, psA, pbA)

            def project(src_full, w_t, m_chunks, L, pmm, evict):
                """evict(psum, m, t) over sum_c w_t[:,c,m-tile].T @ src[:,c,t-tile]."""
                for t in range(L // NT):
                    for m in range(m_chunks):
                        ps = pmm.tile([P, NT], F32, tag="proj")
                        for c in range(DC):
                            nc.tensor.matmul(ps, w_t[:, c, ts(m, P)],
                                             src_full[:, c, ts(t, NT)],
                                             start=(c == 0), stop=(c == DC - 1))
                        evict(ps, m, t)

            def transpose_split(vtile, dst_aug, t, m, ptp):  # vtile [P,512]
                """[P,512] channel-major V tile -> token-major head slabs."""
                for j in range(NT // P):
                    kt = t * (NT // P) + j
                    tp = ptp.tile([P, P], F32R, tag="vtp")
                    nc.tensor.transpose(tp, vtile[:, ts(j, P)], ident_t)
                    nc.vector.tensor_copy(dst_aug[:, kt, 2 * m, 0:DH], tp[:, 0:DH])
                    nc.vector.tensor_copy(dst_aug[:, kt, 2 * m + 1, 0:DH],
                                          tp[:, DH:P])

            def flash(qT_x, khT_x, vh_aug, oT_x, Lkv, psf, pso, fwork, pden):
                """Unnormalized attention + batched denominator reciprocal.

                Scores/attnV matmuls run in same-geometry runs of G so the PE
                doesn't thrash stationary-operand geometry; exp handles two
                score tiles per ACT op; all 16 softmax denominators are
                inverted in one wide DVE reciprocal at the end."""
                KT = Lkv // P
                G = 8
                den = None
                pending = []

                def flush_group():
                    nonlocal den
                    rden = pden.tile([65, NT], F32R, tag="rden", name="rden")
                    with nc.allow_low_precision(reason="softmax denom recip"):
                        nc.vector.reciprocal(rden, den)
                    for off2, hp2, hc2, s2 in pending:
                        r_b = pso.tile([P, NT], F32, tag="r_b")
                        nc.tensor.matmul(r_b, onesq_t[off2:off2 + 1, :],
                                         rden[off2:off2 + 1, :],
                                         start=True, stop=True)
                        nc.vector.tensor_tensor(
                            oT_x[hp2:hp2 + DH, hc2, ts(s2, NT)],
                            oT_x[hp2:hp2 + DH, hc2, ts(s2, NT)],
                            r_b[0:DH, :], OP.mult)
                    pending.clear()
                    den = None

                for h in range(HL):
                    hp, hc = DH * (h % 2), h // 2
                    for s in range(TQ):
                        o_ps = pso.tile([DH + 1, NT], F32, tag="o_ps")
                        for g0 in range(0, KT, G):
                            prl = []
                            for j2 in range(G // 2):
                                sp2 = psf.tile([P, 2, NT], F32, tag="s_ps2")
                                for jj in range(2):
                                    kt = g0 + 2 * j2 + jj
                                    nc.tensor.matmul(
                                        sp2[:, jj, :],
                                        khT_x[hp:hp + DH, hc, ts(kt, P)],
                                        qT_x[hp:hp + DH, hc, ts(s, NT)],
                                        start=True, stop=True,
                                        skip_group_check=True)
                                probs2 = fwork.tile([P, 2, NT], F32R, tag="probs")
                                nc.scalar.activation(probs2, sp2, AF.Exp)
                                prl.append(probs2)
                            for j2 in range(G // 2):
                                for jj in range(2):
                                    kt = g0 + 2 * j2 + jj
                                    nc.tensor.matmul(
                                        o_ps, vh_aug[:, kt, h, :],
                                        prl[j2][:, jj, :],
                                        start=(kt == 0), stop=(kt == KT - 1))
                        idx = h * TQ + s
                        off = (idx % 3) * 32
                        if den is None:
                            den = pden.tile([65, NT], F32, tag="den", name="den")
                        nc.vector.tensor_copy(den[off:off + 1, :],
                                              o_ps[DH:DH + 1, :])
                        nc.vector.tensor_copy(oT_x[hp:hp + DH, hc, ts(s, NT)],
                                              o_ps[0:DH, :])
                        pending.append((off, hp, hc, s))
                        if len(pending) == 3 or idx == HL * TQ - 1:
                            flush_group()

            def out_proj(oT_x, wo_t, pmm, dst_fn):
                for m in range(DC):
                    for t in range(TQ):
                        ps = pmm.tile([P, NT], F32, tag="proj")
                        for hc in range(HC):
                            nc.tensor.matmul(ps, wo_t[:, hc, ts(m, P)],
                                             oT_x[:, hc, ts(t, NT)],
                                             start=(hc == 0), stop=(hc == HC - 1))
                        dst_fn(ps, m, t)

            # ---------------- Phase B: self-attention ----------------
            sattn = ExitStack()
            with sattn:
                sawp = sattn.enter_context(tc.tile_pool(name="saw", bufs=1))
                sap = sattn.enter_context(tc.tile_pool(name="sattn", bufs=1))

                wq_sa_t = sawp.tile([P, DC, HD], F32R, tag="wq")
                nc.sync.dma_start(wq_sa_t, cm(wq_sa))
                wk_sa_t = sawp.tile([P, DC, HD], F32R, tag="wk")
                nc.sync.dma_start(wk_sa_t, cm(wk_sa))
                wv_sa_t = sawp.tile([P, DC, HD], F32R, tag="wv")
                nc.sync.dma_start(wv_sa_t, cm(wv_sa))

                qTs = sap.tile([P, HC, LQ], F32R, tag="qTs")
                khTs = sap.tile([P, HC, LQ], F32R, tag="khTs")
                vh_sa = sap.tile([P, LQ // P, HL, DH + 1], F32R, tag="vh_sa")

                with tc.tile_pool(name="psB1", bufs=2, space="PSUM") as psB1, \
                     tc.tile_pool(name="ptB1", bufs=2, space="PSUM") as ptB1, \
                     tc.tile_pool(name="wkB1", bufs=2) as wkB1:
                    project(xh1, wq_sa_t, HC, LQ, psB1,
                            lambda ps, m, t: nc.vector.tensor_copy(
                                qTs[:, m, ts(t, NT)], ps))
                    project(xh1, wk_sa_t, HC, LQ, psB1,
                            lambda ps, m, t: nc.vector.tensor_copy(
                                khTs[:, m, ts(t, NT)], ps))

                    def v_sa_evict(ps, m, t):
                        vtile = wkB1.tile([P, NT], F32R, tag="vtile")
                        nc.vector.tensor_copy(vtile, ps)
                        transpose_split(vtile, vh_sa, t, m, ptB1)

                    project(xh1, wv_sa_t, HC, LQ, psB1, v_sa_evict)
                    for kt in range(LQ // P):
                        nc.sync.dma_start(vh_sa[:, kt, :, DH], ones4)

                oTs = sap.tile([P, HC, LQ], F32R, tag="oTs")
                with tc.tile_pool(name="psf", bufs=2, space="PSUM") as psf, \
                     tc.tile_pool(name="pso", bufs=2, space="PSUM") as pso, \
                     tc.tile_pool(name="fwk", bufs=4) as fwk, \
                     tc.tile_pool(name="pden", bufs=2) as pden:
                    flash(qTs, khTs, vh_sa, oTs, LQ, psf, pso, fwk, pden)

                wo_sa_t = sawp.tile([P, HC, D], F32R, tag="wq")
                nc.sync.dma_start(wo_sa_t, cm(wo_sa))

                def ar1_dst(ps, m, t):
                    stage = wkB3.tile([P, NT], F32R, tag="ar1st")
                    nc.vector.scalar_tensor_tensor(
                        stage, xh1[:, m, ts(t, NT)], resg_sa_t[:, m:m + 1], ps,
                        OP.mult, OP.add)
                    nc.sync.dma_start(ar1_in_v[:, m, ts(t, NT)], stage)

                with tc.tile_pool(name="psB3", bufs=2, space="PSUM") as psB3, \
                     tc.tile_pool(name="wkB3", bufs=2) as wkB3:
                    out_proj(oTs, wo_sa_t, psB3, ar1_dst)

            selfblk.close()   # frees xh1
            nc.gpsimd.collective_compute(
                "AllReduce", OP.add, replica_groups=RG,
                ins=[ar1_in], outs=[ar1_out])

            # ------------- Phase B': cross K/V (overlaps AR1) -------------
            cattn = ExitStack()
            with cattn:
                cawp = cattn.enter_context(tc.tile_pool(name="caw", bufs=1))
                cap = cattn.enter_context(tc.tile_pool(name="cattn", bufs=1))

                wk_ca_t = cawp.tile([P, DC, HD], F32R, tag="wkc")
                nc.sync.dma_start(wk_ca_t, cm(wk_ca))
                wv_ca_t = cawp.tile([P, DC, HD], F32R, tag="wvc")
                nc.sync.dma_start(wv_ca_t, cm(wv_ca))

                khTc = cap.tile([P, HC, LKV], F32R, tag="khTc")
                vh_ca = cap.tile([P, LKV // P, HL, DH + 1], F32R, tag="vh_ca")

                with tc.tile_pool(name="kvblk", bufs=2) as kvp, \
                     tc.tile_pool(name="psBp", bufs=1, space="PSUM") as psBp, \
                     tc.tile_pool(name="pbBp", bufs=1, space="PSUM") as pbBp, \
                     tc.tile_pool(name="pjBp", bufs=2, space="PSUM") as pjBp, \
                     tc.tile_pool(name="ptBp", bufs=2, space="PSUM") as ptBp, \
                     tc.tile_pool(name="wkBp", bufs=2) as wkBp:

                    def fused_ln_project(src_v, w_t, wfs_t, t, dst_evict):
                        blk = kvp.tile([P, DC, NT], F32R, tag="kvb")
                        nc.sync.dma_start(blk, src_v[:, :, ts(t, NT)])
                        mu, rstd = ln_stats(blk, DC, psBp)
                        negmr = statp.tile([1, NT], F32R, tag="negmr")
                        nc.vector.tensor_mul(negmr, mu, rstd)
                        nc.vector.tensor_scalar_mul(negmr, negmr, -1.0)
                        a_ps = bcast(rstd, pbBp, "bc0")
                        c_ps = bcast(negmr, pbBp, "bc1")
                        a_sb = wkBp.tile([P, NT], F32, tag="a_sb")
                        nc.vector.tensor_copy(a_sb, a_ps)
                        c_sb = wkBp.tile([P, NT], F32, tag="c_sb")
                        nc.vector.tensor_copy(c_sb, c_ps)
                        for m in range(HC):
                            ps = pjBp.tile([P, NT], F32, tag="proj")
                            for c in range(DC):
                                nc.tensor.matmul(ps, w_t[:, c, ts(m, P)],
                                                 blk[:, c, :],
                                                 start=(c == 0), stop=(c == DC - 1))
                            t1 = wkBp.tile([P, NT], F32, tag="t1")
                            nc.vector.tensor_mul(t1, ps, a_sb)
                            dst_evict(t1, c_sb, wfs_t, m, t)

                    def kh_evict(t1, c_sb, wfs_t, m, t):
                        nc.vector.scalar_tensor_tensor(
                            khTc[:, m, ts(t, NT)], c_sb, wfs_t[:, m:m + 1], t1,
                            OP.mult, OP.add)

                    def vh_evict(t1, c_sb, wfs_t, m, t):
                        vtile = wkBp.tile([P, NT], F32R, tag="vtile")
                        nc.vector.scalar_tensor_tensor(
                            vtile, c_sb, wfs_t[:, m:m + 1], t1, OP.mult, OP.add)
                        transpose_split(vtile, vh_ca, t, m, ptBp)

                    for t in range(TKV):
                        fused_ln_project(kT_v, wk_ca_t, wfs_k_t, t, kh_evict)
                        fused_ln_project(vT_v, wv_ca_t, wfs_v_t, t, vh_evict)
                    for kt in range(LKV // P):
                        nc.sync.dma_start(vh_ca[:, kt, :, DH], ones4)

                # ------------- Phase C: q2 LN + cross Q proj (spill xh2) ---------
                wq_ca_t = cawp.tile([P, DC, HD], F32R, tag="wqc")
                nc.sync.dma_start(wq_ca_t, cm(wq_ca))
                qTc = cap.tile([P, HC, LQ], F32R, tag="qTc")
                with tc.tile_pool(name="q2raw", bufs=1) as q2p, \
                     tc.tile_pool(name="psC", bufs=1, space="PSUM") as psC, \
                     tc.tile_pool(name="pbC", bufs=1, space="PSUM") as pbC, \
                     tc.tile_pool(name="pjC", bufs=2, space="PSUM") as pjC:
                    for t in range(TQ):
                        blk = q2p.tile([P, DC, NT], F32R, tag="q2blk")
                        nc.sync.dma_start(blk, ar1_out_v[:, :, ts(t, NT)])
                        xh2b = q2p.tile([P, DC, NT], F32R, tag="xh2b")
                        ln_materialize(blk, xh2b, slice(0, NT), DC, psC, pbC)
                        nc.sync.dma_start(xh2_v[:, :, ts(t, NT)], xh2b)
                        for m in range(HC):
                            ps = pjC.tile([P, NT], F32, tag="proj")
                            for c in range(DC):
                                nc.tensor.matmul(ps, wq_ca_t[:, c, ts(m, P)],
                                                 xh2b[:, c, :],
                                                 start=(c == 0), stop=(c == DC - 1))
                            nc.vector.tensor_copy(qTc[:, m, ts(t, NT)], ps)

                # ------------- Phase D: cross flash + out-proj -> RS2 -------------
                oTc = cap.tile([P, HC, LQ], F32R, tag="oTc")
                with tc.tile_pool(name="psfD", bufs=2, space="PSUM") as psfD, \
                     tc.tile_pool(name="psoD", bufs=2, space="PSUM") as psoD, \
                     tc.tile_pool(name="fwkD", bufs=4) as fwkD, \
                     tc.tile_pool(name="pdenD", bufs=2) as pdenD:
                    flash(qTc, khTc, vh_ca, oTc, LKV, psfD, psoD, fwkD, pdenD)

                wo_ca_t = cawp.tile([P, HC, D], F32R, tag="wkc")
                nc.sync.dma_start(wo_ca_t, cm(wo_ca))

                with tc.tile_pool(name="psD2", bufs=2, space="PSUM") as psD2, \
                     tc.tile_pool(name="wkD2", bufs=2) as wkD2:
                    def rs2_dst(ps, m, t):
                        xh2_t = wkD2.tile([P, NT], F32R, tag="xh2r")
                        nc.sync.dma_start(xh2_t, xh2_v[:, m, ts(t, NT)])
                        stage = wkD2.tile([P, NT], F32R, tag="rs2st")
                        nc.vector.scalar_tensor_tensor(
                            stage, xh2_t, resg_ca_t[:, m:m + 1], ps, OP.mult, OP.add)
                        nc.sync.dma_start(rs2_in_v[:, t, m, :], stage)

                    out_proj(oTc, wo_ca_t, psD2, rs2_dst)

            nc.gpsimd.collective_compute(
                "ReduceScatter", OP.add, replica_groups=RG,
                ins=[rs2_in], outs=[rs2_out])

            # ---------------- Phase E: FFN on the 512-token slice ----------------
            ffn = ExitStack()
            with ffn:
                fp = ffn.enter_context(tc.tile_pool(name="ffn", bufs=1))
                w1p = ffn.enter_context(tc.tile_pool(name="w1p", bufs=4))
                w2p = ffn.enter_context(tc.tile_pool(name="w2p", bufs=2))
                psE = ffn.enter_context(tc.tile_pool(name="psE", bufs=1, space="PSUM"))
                pbE = ffn.enter_context(tc.tile_pool(name="pbE", bufs=1, space="PSUM"))
                pjE = ffn.enter_context(tc.tile_pool(name="pjE", bufs=3, space="PSUM"))
                ework = ffn.enter_context(tc.tile_pool(name="ework", bufs=2))

                x3 = fp.tile([P, DC, RT], F32R, tag="x3")
                nc.sync.dma_start(x3, rs2_out_v)
                xh3 = fp.tile([P, DC, RT], BF16, tag="xh3")
                ln_materialize(x3, xh3, slice(0, RT), DC, psE, pbE)

                hT = fp.tile([P, DFC, RT], BF16, tag="hT")
                for m in range(DFC):
                    w1t = w1p.tile([P, DC, P], BF16, tag="w1t")
                    nc.sync.dma_start(w1t, w1_v[:, :, ts(m, P)])
                    ps = pjE.tile([P, RT], F32, tag="proj")
                    for c in range(DC):
                        nc.tensor.matmul(ps, w1t[:, c, :], xh3[:, c, :],
                                         start=(c == 0), stop=(c == DC - 1))
                    nc.scalar.activation(hT[:, m, :], ps, AF.Relu,
                                         bias=b1f_t[:, m:m + 1])

                for m2 in range(DC):
                    w2t = w2p.tile([P, DFC, P], BF16, tag="w2t")
                    nc.sync.dma_start(w2t, w2_v[:, :, ts(m2, P)])
                    ps = pjE.tile([P, RT], F32, tag="proj")
                    for c in range(DFC):
                        nc.tensor.matmul(ps, w2t[:, c, :], hT[:, c, :],
                                         start=(c == 0), stop=(c == DFC - 1))
                    stage = ework.tile([P, RT], F32, tag="outst")
                    nc.vector.scalar_tensor_tensor(
                        stage, ps, b2f_t[:, m2:m2 + 1], x3[:, m2, :],
                        OP.add, OP.add)
                    nc.sync.dma_start(outT_v[:, m2, :], stage)

    nc.compile()
    return nc


def _host_inputs(inputs):
    """Build the 8 per-core input maps from the full problem inputs."""
    f = np.float32
    q = np.asarray(inputs["q"], f)
    k = np.asarray(inputs["k"], f)
    v = np.asarray(inputs["v"], f)
    g_sa = np.asarray(inputs["g_sa"], f)
    b_sa = np.asarray(inputs["b_sa"], f)
    g_qca = np.asarray(inputs["g_qca"], f)
    b_qca = np.asarray(inputs["b_qca"], f)
    g_kca = np.asarray(inputs["g_kca"], f)
    b_kca = np.asarray(inputs["b_kca"], f)
    g_vca = np.asarray(inputs["g_vca"], f)
    b_vca = np.asarray(inputs["b_vca"], f)
    g_out = np.asarray(inputs["g_out"], f)
    b_out = np.asarray(inputs["b_out"], f)
    for name, b in [("b_sa", b_sa), ("b_qca", b_qca), ("b_kca", b_kca),
                    ("b_vca", b_vca)]:
        assert np.all(b == 0), f"{name} nonzero not supported"

    sa_wq = np.asarray(inputs["sa_wq"], f) * g_sa[None, :] * 0.125
    sa_wk = np.asarray(inputs["sa_wk"], f) * g_sa[None, :]
    sa_wv = np.asarray(inputs["sa_wv"], f) * g_sa[None, :]
    sa_wo = np.asarray(inputs["sa_wo"], f)
    ca_wq = np.asarray(inputs["ca_wq"], f) * g_qca[None, :] * 0.125
    ca_wk = np.asarray(inputs["ca_wk"], f) * g_kca[None, :]
    ca_wv = np.asarray(inputs["ca_wv"], f) * g_vca[None, :]
    ca_wo = np.asarray(inputs["ca_wo"], f)
    import ml_dtypes
    bf = ml_dtypes.bfloat16
    w1f = np.ascontiguousarray(
        (np.asarray(inputs["ffn_w1"], f) * g_out[None, :]).T.astype(bf))
    w2f = np.ascontiguousarray(np.asarray(inputs["ffn_w2"], f).T.astype(bf))
    b1f_vec = np.asarray(inputs["ffn_w1"], f) @ b_out + np.asarray(inputs["ffn_b1"], f)
    b2_vec = np.asarray(inputs["ffn_b2"], f)

    def chunked(vec):  # [C*P] -> [P, C]
        return np.ascontiguousarray(vec.reshape(-1, P).T)

    in_maps = []
    for c in range(8):
        g, r = c // 4, c % 4
        hs = slice(r * HD, (r + 1) * HD)
        flag = 1.0 if r == 0 else 0.0
        wk_eff = ca_wk[hs]
        wv_eff = ca_wv[hs]
        in_maps.append({
            "qT": np.ascontiguousarray(q[g].T),
            "kT": np.ascontiguousarray(k[g].T),
            "vT": np.ascontiguousarray(v[g].T),
            "wq_sa": np.ascontiguousarray(sa_wq[hs].T),
            "wk_sa": np.ascontiguousarray(sa_wk[hs].T),
            "wv_sa": np.ascontiguousarray(sa_wv[hs].T),
            "wo_sa": np.ascontiguousarray(sa_wo[:, hs].T),
            "wq_ca": np.ascontiguousarray(ca_wq[hs].T),
            "wk_ca": np.ascontiguousarray(wk_eff.T),
            "wv_ca": np.ascontiguousarray(wv_eff.T),
            "wo_ca": np.ascontiguousarray(ca_wo[:, hs].T),
            "w1": w1f,
            "w2": w2f,
            "b1f": chunked(b1f_vec),
            "b2f": chunked(b2_vec),
            "resg_sa": chunked(flag * g_sa),
            "resg_ca": chunked(flag * g_qca),
            "wfs_k": chunked(wk_eff.sum(axis=1)),
            "wfs_v": chunked(wv_eff.sum(axis=1)),
            "ones4": np.ones((P, HL), f),
            "onesr": np.ones((1, P), f),
            "onesq": np.ones((65, P), f),
            "ident": np.eye(P, dtype=f),
        })
    return in_maps


def _run(inputs, trace=False, trace_cores=None):
    if "nc" not in _CACHE:
        _CACHE["nc"] = _build()
    nc = _CACHE["nc"]
    in_maps = _host_inputs(inputs)
    res = run_bass_kernel_spmd(nc, in_maps, core_ids=list(range(8)),
                               trace=trace, trace_cores=trace_cores)
    out = np.empty((2, LQ, D), np.float32)
    for c in range(8):
        g, r = c // 4, c % 4
        out[g, r * RT:(r + 1) * RT, :] = res.results[c]["outT"].T
    return out, res


def kernel(**inputs):
    out, _ = _run(inputs)
    return out


# revision 18
# speedup vs baseline: 1.3262x; 1.1642x over previous
"""Trainium2 Bass kernel for nn_DecoderLayer (self-attn + cross-attn + FFN).

Distribution over 8 NeuronCores: data-parallel over batch (B=2) x
tensor-parallel over heads / rows (4-way). Core c handles batch g=c//4 with
group rank r=c%4 owning heads [4r:4r+4]. One AllReduce after self-attn
out-proj, one ReduceScatter after cross-attn out-proj (each within the
4-core group); the FFN then runs row-parallel on each core's 512-token
slice with full weights, so no further collectives are needed.

Everything on-chip is channel-major ([channel-partition, token] layout);
the host transposes inputs/outputs and pre-transposes/slices the weights.
LayerNorm statistics are computed with ones-vector matmuls on the
TensorEngine (partition-axis reduction); softmax runs on transposed scores
(kv on partitions) so no max-subtraction or probability transposes are
needed; the softmax denominator comes from an extra ones-column appended
to V. All matmuls use the float32r (TF32-like) PE path.
"""
import numpy as np
from contextlib import ExitStack

import concourse.bass as bass
import concourse.tile as tile
from concourse import bacc, mybir
from concourse.bass_utils import run_bass_kernel_spmd

F32 = mybir.dt.float32
F32R = mybir.dt.float32r
BF16 = mybir.dt.bfloat16
AF = mybir.ActivationFunctionType
OP = mybir.AluOpType

P = 128
D = 1024          # model dim
DC = D // P       # 8 channel chunks
LQ = 2048         # query length
LKV = 4096        # kv length
HL = 4            # local heads per core
DH = 64           # head dim
HD = HL * DH      # 256 local projection width
HC = HD // P      # 2 chunks
DF = 4096         # FFN hidden
DFC = DF // P     # 32
NT = 512          # token tile
TQ = LQ // NT     # 4
TKV = LKV // NT   # 8
RT = 512          # per-rank token slice after RS
EPS = 1e-5
RG = [[0, 1, 2, 3], [4, 5, 6, 7]]

_CACHE = {}


def _build():
    nc = bacc.Bacc("TRN2", target_bir_lowering=False, debug=False, num_devices=8)

    def din(name, shape, dt=F32R):
        return nc.dram_tensor(name, shape, dt, kind="ExternalInput").ap()

    qT = din("qT", [D, LQ])
    kT = din("kT", [D, LKV], BF16)
    vT = din("vT", [D, LKV], BF16)
    wq_sa = din("wq_sa", [D, HD], BF16)
    wk_sa = din("wk_sa", [D, HD], BF16)
    wv_sa = din("wv_sa", [D, HD], BF16)
    wo_sa = din("wo_sa", [HD, D], BF16)
    wq_ca = din("wq_ca", [D, HD], BF16)
    wk_ca = din("wk_ca", [D, HD], BF16)
    wv_ca = din("wv_ca", [D, HD], BF16)
    wo_ca = din("wo_ca", [HD, D], BF16)
    w1 = din("w1", [D, DF], BF16)
    w2 = din("w2", [DF, D], BF16)
    b1f = din("b1f", [P, DFC], F32)
    b2f = din("b2f", [P, DC], F32)
    resg_sa = din("resg_sa", [P, DC], F32)
    resg_ca = din("resg_ca", [P, DC], F32)
    wfs_k = din("wfs_k", [P, HC], F32)
    wfs_v = din("wfs_v", [P, HC], F32)
    ones4 = din("ones4", [P, HL])
    ones4b = din("ones4b", [P, HL], BF16)
    onesr = din("onesr", [1, P])
    onesq = din("onesq", [65, P])
    ident = din("ident", [P, P], BF16)

    outT = nc.dram_tensor("outT", [D, RT], F32, kind="ExternalOutput").ap()

    # internal DRAM
    ar1_in = nc.dram_tensor("ar1_in", [D, LQ], F32R).ap()
    ar1_out = nc.dram_tensor("ar1_out", [D, LQ], F32R).ap()
    xh2_d = nc.dram_tensor("xh2_d", [D, LQ], F32R).ap()
    rs2_in = nc.dram_tensor("rs2_in", [TQ, D, NT], F32R).ap()
    rs2_out = nc.dram_tensor("rs2_out", [D, RT], F32R).ap()

    def cm(ap):  # [C*P, L] -> [P, C, L]
        return ap.rearrange("(c p) l -> p c l", p=P)

    qT_v, kT_v, vT_v = cm(qT), cm(kT), cm(vT)
    ar1_in_v, ar1_out_v, xh2_v = cm(ar1_in), cm(ar1_out), cm(xh2_d)
    rs2_in_v = rs2_in.rearrange("t (c p) l -> p t c l", p=P)
    rs2_out_v, outT_v = cm(rs2_out), cm(outT)
    w1_v = cm(w1)   # [P, 8, 4096]
    w2_v = cm(w2)   # [P, 32, 1024]

    ts = bass.ts

    with tile.TileContext(nc) as tc:
        ctx = ExitStack()
        with ctx:
            constp = ctx.enter_context(tc.tile_pool(name="const", bufs=1))
            statp1 = ctx.enter_context(tc.tile_pool(name="stats1", bufs=1))
            statp = ctx.enter_context(tc.tile_pool(name="stats2", bufs=2))
            work = ctx.enter_context(tc.tile_pool(name="work", bufs=2))

            ones4_t = constp.tile([P, HL], F32R)
            nc.sync.dma_start(ones4_t, ones4)
            ones4b_t = constp.tile([P, HL], BF16)
            nc.sync.dma_start(ones4b_t, ones4b)
            onesr_t = constp.tile([1, P], F32R)
            nc.sync.dma_start(onesr_t, onesr)
            onesq_t = constp.tile([65, P], F32R)
            nc.sync.dma_start(onesq_t, onesq)
            ident_t = constp.tile([P, P], BF16)
            nc.sync.dma_start(ident_t, ident)
            eps_t = constp.tile([1, 1], F32)
            nc.vector.memset(eps_t, EPS)
            b1f_t = constp.tile([P, DFC], F32)
            nc.sync.dma_start(b1f_t, b1f)
            b2f_t = constp.tile([P, DC], F32)
            nc.sync.dma_start(b2f_t, b2f)
            resg_sa_t = constp.tile([P, DC], F32)
            nc.sync.dma_start(resg_sa_t, resg_sa)
            resg_ca_t = constp.tile([P, DC], F32)
            nc.sync.dma_start(resg_ca_t, resg_ca)
            wfs_k_t = constp.tile([P, HC], F32)
            nc.sync.dma_start(wfs_k_t, wfs_k)
            wfs_v_t = constp.tile([P, HC], F32)
            nc.sync.dma_start(wfs_v_t, wfs_v)

            def ln_stats(blk, C, pst):
                """blk [P, C, 512] -> (mu [1,512] F32R, rstd [1,512] F32R)."""
                bf = blk.dtype == BF16
                ones_t = ones4b_t if bf else ones4_t
                xdt = BF16 if bf else F32R
                ps_sum = pst.tile([1, NT], F32, tag="ps_sum")
                ps_sq = pst.tile([1, NT], F32, tag="ps_sq")
                for c in range(C):
                    xsq = work.tile([P, NT], xdt, tag="xsq")
                    nc.vector.tensor_mul(xsq, blk[:, c, :], blk[:, c, :])
                    nc.tensor.matmul(ps_sum, ones_t[:, 0:1], blk[:, c, :],
                                     start=(c == 0), stop=(c == C - 1))
                    nc.tensor.matmul(ps_sq, ones_t[:, 0:1], xsq,
                                     start=(c == 0), stop=(c == C - 1))
                mu = statp.tile([1, NT], F32R, tag="mu")
                with nc.allow_low_precision(reason="ln moments f32r"):
                    nc.vector.tensor_scalar_mul(mu, ps_sum, 1.0 / D)
                ex2 = statp1.tile([1, NT], F32, tag="ex2")
                nc.vector.tensor_scalar_mul(ex2, ps_sq, 1.0 / D)
                musq = statp1.tile([1, NT], F32, tag="musq")
                nc.vector.tensor_mul(musq, mu, mu)
                var = statp1.tile([1, NT], F32, tag="var")
                nc.vector.tensor_tensor(var, ex2, musq, OP.subtract)
                std = statp1.tile([1, NT], F32, tag="std")
                nc.scalar.activation(std, var, AF.Sqrt, bias=eps_t)
                rstd = statp.tile([1, NT], F32R, tag="rstd")
                with nc.allow_low_precision(reason="rstd in f32r for PE bcast"):
                    nc.vector.reciprocal(rstd, std)
                return mu, rstd

            def bcast(row, pool, tag):
                """[1,512] F32R -> psum [P,512] F32 replicated across partitions."""
                ps = pool.tile([P, NT], F32, tag=tag)
                nc.tensor.matmul(ps, onesr_t, row, start=True, stop=True)
                return ps

            def ln_materialize(blk, dst, tslice, C, pst, pbc, dst_bf=None):
                """dst[:, c, tslice] = (blk - mu) * rstd (channel-major LN)."""
                mu, rstd = ln_stats(blk, C, pst)
                mu_b = bcast(mu, pbc, "bc0")
                rstd_b = bcast(rstd, pbc, "bc1")
                for c in range(C):
                    tmp = work.tile([P, NT], F32, tag="lnt")
                    nc.vector.tensor_tensor(tmp, blk[:, c, :], mu_b, OP.subtract)
                    nc.vector.tensor_tensor(dst[:, c, tslice], tmp, rstd_b, OP.mult)
                    if dst_bf is not None:
                        nc.scalar.copy(dst_bf[:, c, tslice], dst[:, c, tslice])

            # ---------------- Phase A: LN(q) -> xh1 (resident A..B3) -------------
            selfblk = ExitStack()
            ctx.enter_context(selfblk)
            bigp = selfblk.enter_context(tc.tile_pool(name="big", bufs=1))
            xh1 = bigp.tile([P, DC, LQ], F32R, tag="xh1")
            xh1b = bigp.tile([P, DC, LQ], BF16, tag="xh1b")
            with tc.tile_pool(name="qraw", bufs=2) as qrawp, \
                 tc.tile_pool(name="psA", bufs=1, space="PSUM") as psA, \
                 tc.tile_pool(name="pbA", bufs=1, space="PSUM") as pbA:
                for t in range(TQ):
                    blk = qrawp.tile([P, DC, NT], F32R, tag="qblk")
                    nc.sync.dma_start(blk, qT_v[:, :, ts(t, NT)])
                    ln_materialize(blk, xh1, ts(t, NT), DC, psA, pbA, dst_bf=xh1b)

            def project(src_full, w_t, m_chunks, L, pmm, evict):
                """evict(psum, m, t) over sum_c w_t[:,c,m-tile].T @ src[:,c,t-tile]."""
                for t in range(L // NT):
                    for m in range(m_chunks):
                        ps = pmm.tile([P, NT], F32, tag="proj")
                        for c in range(DC):
                            nc.tensor.matmul(ps, w_t[:, c, ts(m, P)],
                                             src_full[:, c, ts(t, NT)],
                                             start=(c == 0), stop=(c == DC - 1))
                        evict(ps, m, t)

            def transpose_split(vtile, dst_aug, t, m, ptp):  # vtile [P,512]
                """[P,512] channel-major V tile -> token-major head slabs."""
                for j in range(NT // P):
                    kt = t * (NT // P) + j
                    tp = ptp.tile([P, P], BF16, tag="vtp")
                    nc.tensor.transpose(tp, vtile[:, ts(j, P)], ident_t)
                    nc.vector.tensor_copy(dst_aug[:, kt, 2 * m, 0:DH], tp[:, 0:DH])
                    nc.vector.tensor_copy(dst_aug[:, kt, 2 * m + 1, 0:DH],
                                          tp[:, DH:P])

            def flash(qT_x, khT_x, vh_aug, oT_x, Lkv, psf, pso, fwork, pden):
                """Unnormalized attention + batched denominator reciprocal.

                Scores/attnV matmuls run in same-geometry runs of G so the PE
                doesn't thrash stationary-operand geometry; exp handles two
                score tiles per ACT op; all 16 softmax denominators are
                inverted in one wide DVE reciprocal at the end."""
                KT = Lkv // P
                G = 8
                den = None
                pending = []

                def flush_group():
                    nonlocal den
                    rden = pden.tile([65, NT], F32R, tag="rden", name="rden")
                    with nc.allow_low_precision(reason="softmax denom recip"):
                        nc.vector.reciprocal(rden, den)
                    for off2, hp2, hc2, s2 in pending:
                        r_b = pso.tile([P, NT], F32, tag="r_b")
                        nc.tensor.matmul(r_b, onesq_t[off2:off2 + 1, :],
                                         rden[off2:off2 + 1, :],
                                         start=True, stop=True)
                        nc.vector.tensor_tensor(
                            oT_x[hp2:hp2 + DH, hc2, ts(s2, NT)],
                            oT_x[hp2:hp2 + DH, hc2, ts(s2, NT)],
                            r_b[0:DH, :], OP.mult)
                    pending.clear()
                    den = None

                for h in range(HL):
                    hp, hc = DH * (h % 2), h // 2
                    for s in range(TQ):
                        o_ps = pso.tile([DH + 1, NT], F32, tag="o_ps")
                        for g0 in range(0, KT, G):
                            prl = []
                            for j2 in range(G // 2):
                                sp2 = psf.tile([P, 2, NT], F32, tag="s_ps2")
                                for jj in range(2):
                                    kt = g0 + 2 * j2 + jj
                                    nc.tensor.matmul(
                                        sp2[:, jj, :],
                                        khT_x[hp:hp + DH, hc, ts(kt, P)],
                                        qT_x[hp:hp + DH, hc, ts(s, NT)],
                                        start=True, stop=True,
                                        skip_group_check=True)
                                probs2 = fwork.tile([P, 2, NT], BF16, tag="probs")
                                nc.scalar.activation(probs2, sp2, AF.Exp)
                                prl.append(probs2)
                            for j2 in range(G // 2):
                                for jj in range(2):
                                    kt = g0 + 2 * j2 + jj
                                    nc.tensor.matmul(
                                        o_ps, vh_aug[:, kt, h, :],
                                        prl[j2][:, jj, :],
                                        start=(kt == 0), stop=(kt == KT - 1))
                        idx = h * TQ + s
                        off = (idx % 3) * 32
                        if den is None:
                            den = pden.tile([65, NT], F32, tag="den", name="den")
                        nc.vector.tensor_copy(den[off:off + 1, :],
                                              o_ps[DH:DH + 1, :])
                        nc.vector.tensor_copy(oT_x[hp:hp + DH, hc, ts(s, NT)],
                                              o_ps[0:DH, :])
                        pending.append((off, hp, hc, s))
                        if len(pending) == 3 or idx == HL * TQ - 1:
                            flush_group()

            def out_proj(oT_x, wo_t, pmm, dst_fn):
                for m in range(DC):
                    for t in range(TQ):
                        ps = pmm.tile([P, NT], F32, tag="proj")
                        for hc in range(HC):
                            nc.tensor.matmul(ps, wo_t[:, hc, ts(m, P)],
                                             oT_x[:, hc, ts(t, NT)],
                                             start=(hc == 0), stop=(hc == HC - 1))
                        dst_fn(ps, m, t)

            # ---------------- Phase B: self-attention ----------------
            sattn = ExitStack()
            with sattn:
                sawp = sattn.enter_context(tc.tile_pool(name="saw", bufs=1))
                sap = sattn.enter_context(tc.tile_pool(name="sattn", bufs=1))

                wq_sa_t = sawp.tile([P, DC, HD], BF16, tag="wq")
                nc.sync.dma_start(wq_sa_t, cm(wq_sa))
                wk_sa_t = sawp.tile([P, DC, HD], BF16, tag="wk")
                nc.sync.dma_start(wk_sa_t, cm(wk_sa))
                wv_sa_t = sawp.tile([P, DC, HD], BF16, tag="wv")
                nc.sync.dma_start(wv_sa_t, cm(wv_sa))

                qTs = sap.tile([P, HC, LQ], BF16, tag="qTs")
                khTs = sap.tile([P, HC, LQ], BF16, tag="khTs")
                vh_sa = sap.tile([P, LQ // P, HL, DH + 1], BF16, tag="vh_sa")

                with tc.tile_pool(name="psB1", bufs=2, space="PSUM") as psB1, \
                     tc.tile_pool(name="ptB1", bufs=2, space="PSUM") as ptB1, \
                     tc.tile_pool(name="wkB1", bufs=2) as wkB1:
                    project(xh1b, wq_sa_t, HC, LQ, psB1,
                            lambda ps, m, t: nc.vector.tensor_copy(
                                qTs[:, m, ts(t, NT)], ps))
                    project(xh1b, wk_sa_t, HC, LQ, psB1,
                            lambda ps, m, t: nc.vector.tensor_copy(
                                khTs[:, m, ts(t, NT)], ps))

                    def v_sa_evict(ps, m, t):
                        vtile = wkB1.tile([P, NT], BF16, tag="vtile")
                        nc.vector.tensor_copy(vtile, ps)
                        transpose_split(vtile, vh_sa, t, m, ptB1)

                    project(xh1b, wv_sa_t, HC, LQ, psB1, v_sa_evict)
                    for kt in range(LQ // P):
                        nc.sync.dma_start(vh_sa[:, kt, :, DH], ones4b)

                oTs = sap.tile([P, HC, LQ], BF16, tag="oTs")
                with tc.tile_pool(name="psf", bufs=2, space="PSUM") as psf, \
                     tc.tile_pool(name="pso", bufs=2, space="PSUM") as pso, \
                     tc.tile_pool(name="fwk", bufs=4) as fwk, \
                     tc.tile_pool(name="pden", bufs=2) as pden:
                    flash(qTs, khTs, vh_sa, oTs, LQ, psf, pso, fwk, pden)

                wo_sa_t = sawp.tile([P, HC, D], BF16, tag="wq")
                nc.sync.dma_start(wo_sa_t, cm(wo_sa))

                def ar1_dst(ps, m, t):
                    stage = wkB3.tile([P, NT], F32R, tag="ar1st")
                    nc.vector.scalar_tensor_tensor(
                        stage, xh1[:, m, ts(t, NT)], resg_sa_t[:, m:m + 1], ps,
                        OP.mult, OP.add)
                    nc.sync.dma_start(ar1_in_v[:, m, ts(t, NT)], stage)

                with tc.tile_pool(name="psB3", bufs=2, space="PSUM") as psB3, \
                     tc.tile_pool(name="wkB3", bufs=2) as wkB3:
                    out_proj(oTs, wo_sa_t, psB3, ar1_dst)

            selfblk.close()   # frees xh1
            nc.gpsimd.collective_compute(
                "AllReduce", OP.add, replica_groups=RG,
                ins=[ar1_in], outs=[ar1_out])

            # ------------- Phase B': cross K/V (overlaps AR1) -------------
            cattn = ExitStack()
            with cattn:
                cawp = cattn.enter_context(tc.tile_pool(name="caw", bufs=1))
                cap = cattn.enter_context(tc.tile_pool(name="cattn", bufs=1))

                wk_ca_t = cawp.tile([P, DC, HD], BF16, tag="wkc")
                nc.sync.dma_start(wk_ca_t, cm(wk_ca))
                wv_ca_t = cawp.tile([P, DC, HD], BF16, tag="wvc")
                nc.sync.dma_start(wv_ca_t, cm(wv_ca))

                khTc = cap.tile([P, HC, LKV], BF16, tag="khTc")
                vh_ca = cap.tile([P, LKV // P, HL, DH + 1], BF16, tag="vh_ca")

                with tc.tile_pool(name="kvblk", bufs=2) as kvp, \
                     tc.tile_pool(name="psBp", bufs=1, space="PSUM") as psBp, \
                     tc.tile_pool(name="pbBp", bufs=1, space="PSUM") as pbBp, \
                     tc.tile_pool(name="pjBp", bufs=2, space="PSUM") as pjBp, \
                     tc.tile_pool(name="ptBp", bufs=2, space="PSUM") as ptBp, \
                     tc.tile_pool(name="wkBp", bufs=2) as wkBp:

                    def fused_ln_project(src_v, w_t, wfs_t, t, dst_evict):
                        blk = kvp.tile([P, DC, NT], BF16, tag="kvb")
                        nc.sync.dma_start(blk, src_v[:, :, ts(t, NT)])
                        mu, rstd = ln_stats(blk, DC, psBp)
                        negmr = statp.tile([1, NT], F32R, tag="negmr")
                        nc.vector.tensor_mul(negmr, mu, rstd)
                        nc.vector.tensor_scalar_mul(negmr, negmr, -1.0)
                        a_ps = bcast(rstd, pbBp, "bc0")
                        c_ps = bcast(negmr, pbBp, "bc1")
                        a_sb = wkBp.tile([P, NT], F32, tag="a_sb")
                        nc.vector.tensor_copy(a_sb, a_ps)
                        c_sb = wkBp.tile([P, NT], F32, tag="c_sb")
                        nc.vector.tensor_copy(c_sb, c_ps)
                        for m in range(HC):
                            ps = pjBp.tile([P, NT], F32, tag="proj")
                            for c in range(DC):
                                nc.tensor.matmul(ps, w_t[:, c, ts(m, P)],
                                                 blk[:, c, :],
                                                 start=(c == 0), stop=(c == DC - 1))
                            t1 = wkBp.tile([P, NT], F32, tag="t1")
                            nc.vector.tensor_mul(t1, ps, a_sb)
                            dst_evict(t1, c_sb, wfs_t, m, t)

                    def kh_evict(t1, c_sb, wfs_t, m, t):
                        nc.vector.scalar_tensor_tensor(
                            khTc[:, m, ts(t, NT)], c_sb, wfs_t[:, m:m + 1], t1,
                            OP.mult, OP.add)

                    def vh_evict(t1, c_sb, wfs_t, m, t):
                        vtile = wkBp.tile([P, NT], BF16, tag="vtile")
                        nc.vector.scalar_tensor_tensor(
                            vtile, c_sb, wfs_t[:, m:m + 1], t1, OP.mult, OP.add)
                        transpose_split(vtile, vh_ca, t, m, ptBp)

                    for t in range(TKV):
                        fused_ln_project(kT_v, wk_ca_t, wfs_k_t, t, kh_evict)
                        fused_ln_project(vT_v, wv_ca_t, wfs_v_t, t, vh_evict)
                    for kt in range(LKV // P):
                        nc.sync.dma_start(vh_ca[:, kt, :, DH], ones4b)

                # ------------- Phase C: q2 LN + cross Q proj (spill xh2) ---------
                wq_ca_t = cawp.tile([P, DC, HD], BF16, tag="wqc")
                nc.sync.dma_start(wq_ca_t, cm(wq_ca))
                qTc = cap.tile([P, HC, LQ], BF16, tag="qTc")
                with tc.tile_pool(name="q2raw", bufs=1) as q2p, \
                     tc.tile_pool(name="psC", bufs=1, space="PSUM") as psC, \
                     tc.tile_pool(name="pbC", bufs=1, space="PSUM") as pbC, \
                     tc.tile_pool(name="pjC", bufs=2, space="PSUM") as pjC:
                    for t in range(TQ):
                        blk = q2p.tile([P, DC, NT], F32R, tag="q2blk")
                        nc.sync.dma_start(blk, ar1_out_v[:, :, ts(t, NT)])
                        xh2b = q2p.tile([P, DC, NT], F32R, tag="xh2b")
                        xh2bb = q2p.tile([P, DC, NT], BF16, tag="xh2bb")
                        ln_materialize(blk, xh2b, slice(0, NT), DC, psC, pbC,
                                       dst_bf=xh2bb)
                        nc.sync.dma_start(xh2_v[:, :, ts(t, NT)], xh2b)
                        for m in range(HC):
                            ps = pjC.tile([P, NT], F32, tag="proj")
                            for c in range(DC):
                                nc.tensor.matmul(ps, wq_ca_t[:, c, ts(m, P)],
                                                 xh2bb[:, c, :],
                                                 start=(c == 0), stop=(c == DC - 1))
                            nc.vector.tensor_copy(qTc[:, m, ts(t, NT)], ps)

                # ------------- Phase D: cross flash + out-proj -> RS2 -------------
                oTc = cap.tile([P, HC, LQ], BF16, tag="oTc")
                with tc.tile_pool(name="psfD", bufs=2, space="PSUM") as psfD, \
                     tc.tile_pool(name="psoD", bufs=2, space="PSUM") as psoD, \
                     tc.tile_pool(name="fwkD", bufs=4) as fwkD, \
                     tc.tile_pool(name="pdenD", bufs=2) as pdenD:
                    flash(qTc, khTc, vh_ca, oTc, LKV, psfD, psoD, fwkD, pdenD)

                wo_ca_t = cawp.tile([P, HC, D], BF16, tag="wkc")
                nc.sync.dma_start(wo_ca_t, cm(wo_ca))

                with tc.tile_pool(name="psD2", bufs=2, space="PSUM") as psD2, \
                     tc.tile_pool(name="wkD2", bufs=2) as wkD2:
                    def rs2_dst(ps, m, t):
                        xh2_t = wkD2.tile([P, NT], F32R, tag="xh2r")
                        nc.sync.dma_start(xh2_t, xh2_v[:, m, ts(t, NT)])
                        stage = wkD2.tile([P, NT], F32R, tag="rs2st")
                        nc.vector.scalar_tensor_tensor(
                            stage, xh2_t, resg_ca_t[:, m:m + 1], ps, OP.mult, OP.add)
                        nc.sync.dma_start(rs2_in_v[:, t, m, :], stage)

                    out_proj(oTc, wo_ca_t, psD2, rs2_dst)

            nc.gpsimd.collective_compute(
                "ReduceScatter", OP.add, replica_groups=RG,
                ins=[rs2_in], outs=[rs2_out])

            # ---------------- Phase E: FFN on the 512-token slice ----------------
            ffn = ExitStack()
            with ffn:
                fp = ffn.enter_context(tc.tile_pool(name="ffn", bufs=1))
                w1p = ffn.enter_context(tc.tile_pool(name="w1p", bufs=4))
                w2p = ffn.enter_context(tc.tile_pool(name="w2p", bufs=2))
                psE = ffn.enter_context(tc.tile_pool(name="psE", bufs=1, space="PSUM"))
                pbE = ffn.enter_context(tc.tile_pool(name="pbE", bufs=1, space="PSUM"))
                pjE = ffn.enter_context(tc.tile_pool(name="pjE", bufs=3, space="PSUM"))
                ework = ffn.enter_context(tc.tile_pool(name="ework", bufs=2))

                x3 = fp.tile([P, DC, RT], F32R, tag="x3")
                nc.sync.dma_start(x3, rs2_out_v)
                xh3 = fp.tile([P, DC, RT], BF16, tag="xh3")
                ln_materialize(x3, xh3, slice(0, RT), DC, psE, pbE)

                hT = fp.tile([P, DFC, RT], BF16, tag="hT")
                for m in range(DFC):
                    w1t = w1p.tile([P, DC, P], BF16, tag="w1t")
                    nc.sync.dma_start(w1t, w1_v[:, :, ts(m, P)])
                    ps = pjE.tile([P, RT], F32, tag="proj")
                    for c in range(DC):
                        nc.tensor.matmul(ps, w1t[:, c, :], xh3[:, c, :],
                                         start=(c == 0), stop=(c == DC - 1))
                    nc.scalar.activation(hT[:, m, :], ps, AF.Relu,
                                         bias=b1f_t[:, m:m + 1])

                for m2 in range(DC):
                    w2t = w2p.tile([P, DFC, P], BF16, tag="w2t")
                    nc.sync.dma_start(w2t, w2_v[:, :, ts(m2, P)])
                    ps = pjE.tile([P, RT], F32, tag="proj")
                    for c in range(DFC):
                        nc.tensor.matmul(ps, w2t[:, c, :], hT[:, c, :],
                                         start=(c == 0), stop=(c == DFC - 1))
                    stage = ework.tile([P, RT], F32, tag="outst")
                    nc.vector.scalar_tensor_tensor(
                        stage, ps, b2f_t[:, m2:m2 + 1], x3[:, m2, :],
                        OP.add, OP.add)
                    nc.sync.dma_start(outT_v[:, m2, :], stage)

    nc.compile()
    return nc


def _host_inputs(inputs):
    """Build the 8 per-core input maps from the full problem inputs."""
    f = np.float32
    q = np.asarray(inputs["q"], f)
    k = np.asarray(inputs["k"], f)
    v = np.asarray(inputs["v"], f)
    g_sa = np.asarray(inputs["g_sa"], f)
    b_sa = np.asarray(inputs["b_sa"], f)
    g_qca = np.asarray(inputs["g_qca"], f)
    b_qca = np.asarray(inputs["b_qca"], f)
    g_kca = np.asarray(inputs["g_kca"], f)
    b_kca = np.asarray(inputs["b_kca"], f)
    g_vca = np.asarray(inputs["g_vca"], f)
    b_vca = np.asarray(inputs["b_vca"], f)
    g_out = np.asarray(inputs["g_out"], f)
    b_out = np.asarray(inputs["b_out"], f)
    for name, b in [("b_sa", b_sa), ("b_qca", b_qca), ("b_kca", b_kca),
                    ("b_vca", b_vca)]:
        assert np.all(b == 0), f"{name} nonzero not supported"

    sa_wq = np.asarray(inputs["sa_wq"], f) * g_sa[None, :] * 0.125
    sa_wk = np.asarray(inputs["sa_wk"], f) * g_sa[None, :]
    sa_wv = np.asarray(inputs["sa_wv"], f) * g_sa[None, :]
    sa_wo = np.asarray(inputs["sa_wo"], f)
    ca_wq = np.asarray(inputs["ca_wq"], f) * g_qca[None, :] * 0.125
    ca_wk = np.asarray(inputs["ca_wk"], f) * g_kca[None, :]
    ca_wv = np.asarray(inputs["ca_wv"], f) * g_vca[None, :]
    ca_wo = np.asarray(inputs["ca_wo"], f)
    import ml_dtypes
    bf = ml_dtypes.bfloat16
    w1f = np.ascontiguousarray(
        (np.asarray(inputs["ffn_w1"], f) * g_out[None, :]).T.astype(bf))
    w2f = np.ascontiguousarray(np.asarray(inputs["ffn_w2"], f).T.astype(bf))
    b1f_vec = np.asarray(inputs["ffn_w1"], f) @ b_out + np.asarray(inputs["ffn_b1"], f)
    b2_vec = np.asarray(inputs["ffn_b2"], f)

    def chunked(vec):  # [C*P] -> [P, C]
        return np.ascontiguousarray(vec.reshape(-1, P).T)

    in_maps = []
    for c in range(8):
        g, r = c // 4, c % 4
        hs = slice(r * HD, (r + 1) * HD)
        flag = 1.0 if r == 0 else 0.0
        wk_eff = ca_wk[hs]
        wv_eff = ca_wv[hs]
        in_maps.append({
            "qT": np.ascontiguousarray(q[g].T),
            "kT": np.ascontiguousarray(k[g].T.astype(bf)),
            "vT": np.ascontiguousarray(v[g].T.astype(bf)),
            "wq_sa": np.ascontiguousarray(sa_wq[hs].T.astype(bf)),
            "wk_sa": np.ascontiguousarray(sa_wk[hs].T.astype(bf)),
            "wv_sa": np.ascontiguousarray(sa_wv[hs].T.astype(bf)),
            "wo_sa": np.ascontiguousarray(sa_wo[:, hs].T.astype(bf)),
            "wq_ca": np.ascontiguousarray(ca_wq[hs].T.astype(bf)),
            "wk_ca": np.ascontiguousarray(wk_eff.T.astype(bf)),
            "wv_ca": np.ascontiguousarray(wv_eff.T.astype(bf)),
            "wo_ca": np.ascontiguousarray(ca_wo[:, hs].T.astype(bf)),
            "w1": w1f,
            "w2": w2f,
            "b1f": chunked(b1f_vec),
            "b2f": chunked(b2_vec),
            "resg_sa": chunked(flag * g_sa),
            "resg_ca": chunked(flag * g_qca),
            "wfs_k": chunked(wk_eff.sum(axis=1)),
            "wfs_v": chunked(wv_eff.sum(axis=1)),
            "ones4": np.ones((P, HL), f),
            "ones4b": np.ones((P, HL), bf),
            "onesr": np.ones((1, P), f),
            "onesq": np.ones((65, P), f),
            "ident": np.eye(P).astype(bf),
        })
    return in_maps


def _run(inputs, trace=False, trace_cores=None):
    if "nc" not in _CACHE:
        _CACHE["nc"] = _build()
    nc = _CACHE["nc"]
    in_maps = _host_inputs(inputs)
    res = run_bass_kernel_spmd(nc, in_maps, core_ids=list(range(8)),
                               trace=trace, trace_cores=trace_cores)
    out = np.empty((2, LQ, D), np.float32)
    for c in range(8):
        g, r = c // 4, c % 4
        out[g, r * RT:(r + 1) * RT, :] = res.results[c]["outT"].T
    return out, res


def kernel(**inputs):
    out, _ = _run(inputs)
    return out
